# revision 16
# baseline (speedup 1.0000x reference)
"""MultiHeadDenseAttention on 8 Trainium2 NeuronCores.

Head-sharded tensor parallelism: each core computes 2 of 16 heads
(value projection slice, per-head MLP attention logits, softmax, S@V),
then an AllToAll exchanges head-blocks for row-blocks so each core
computes the output projection for its 512 rows with the full Wo.

Schedule (v5): the Activation engine (exp of 16.8M logits/core) is the
hard floor, so everything else is software-pipelined around it:
  - logits matmul in fp8e4m3 DoubleRow mode (2 k-tiles of 64 contracted
    per pass, 0.5 cycles/row): hid stored fp8 [64, 2, 4096], k-tile 1 =
    [ones (fused b2); zeros]; W2 host-scaled by 8, un-scaled in exp.
  - value projection directly in [m, d] layout (lhsT = x chunk, rhs =
    Wv.T block, PSUM accumulation over the 8 feature chunks) in bf16.
  - value/hid for rep r+1 are emitted interleaved between rep r's P2
    blocks (double-buffered vh/hidT by rep parity), so the exp stream
    never waits on them.
  - A2A payload in bf16; P4 (normalize + output projection) of rep r-1
    is emitted between rep r and r+1, hiding each collective behind a
    full rep of compute.

Layouts (per core c, heads 2c / 2c+1):
  xt    [1024, 4096] bf16  x.reshape(4096,1024).T  (feat on partitions)
  xc    [128, 4096] f32r   xt rows [128c, 128c+128) (this core's head cols)
  wv    [128, 1024] bf16   col block f = Wv.T[128f:+128, 128c:+128]
  vh[p][b] [128, 16, 132] bf16  per m-chunk: [64 h0 | one | pad | 64 h1 | one | pad]
  hidT  [64, 2, 4096] fp8  k-tile 0 = relu(W1@x+b1), k-tile 1 = ones/zeros
  w2dr  [64, 2, 2048] fp8  k-tile 0 = 8*W2.T, k-tile 1 row 0 = 8*b2
  S@V: po[65, 512] = vh_aug.T @ expT  (row 64 = sum of exp)
  A2A [8, 65, 512] bf16 x2 heads, normalize after exchange, out = act @ Wo.T.
"""

import sys

if "/opt/trn_rl_repo" not in sys.path:
    sys.path.insert(0, "/opt/trn_rl_repo")

from contextlib import ExitStack

import numpy as np
import ml_dtypes

import bass_rust
import concourse.bass as bass
import concourse.tile as tile
from concourse import mybir
from concourse.bass_utils import run_bass_kernel_spmd

F32 = mybir.dt.float32
F32R = mybir.dt.float32r
BF16 = mybir.dt.bfloat16
F8 = mybir.dt.float8e4
AF = mybir.ActivationFunctionType

NC = 8            # cores
B = 2             # batch
N_SEQ = 2048      # seq len == max_seq_len (m)
FEAT = 1024
H = 16            # heads
D = 64            # head dim
NTOT = B * N_SEQ  # 4096 flattened rows
NBLK = 512        # n-block size
NB = NTOT // NBLK # 8 n-blocks (== A2A shards == cores)
MC = N_SEQ // 128 # 16 m-chunks per batch
W2SCALE = 8.0     # host-side W2 scaling into fp8 range; undone in exp


def _split_sem_waits(nc, limit=1):
    """Walrus rejects instructions with more than ~1 sync wait; move the
    excess onto NOPs on the same engine inserted immediately before."""
    blocks = {}
    for f in nc.m.functions:
        for bb in f.blocks:
            blocks[bb.name] = bb
    for bb in blocks.values():
        i = 0
        while i < len(bb.instructions):
            inst = bb.instructions[i]
            si = inst.sync_info
            if si is not None and si.on_wait and len(si.on_wait) > limit:
                waits = list(si.on_wait)
                chunks = [waits[j : j + limit] for j in range(0, len(waits), limit)]
                si.on_wait = chunks[-1]
                engine = nc.engines[inst.engine]
                for chunk in chunks[:-1]:
                    d = engine.nop(nofuse=True, hint="wait_split")
                    dinst = d.ins if hasattr(d, "ins") else d
                    for ob in blocks.values():
                        if ob.instructions and ob.instructions[-1] is dinst:
                            ob.instructions.pop()
                            break
                    dinst.sync_info = bass_rust.SyncInfo(on_wait=chunk, on_update=[])
                    bb.instructions.insert(i, dinst)
                    i += 1
            i += 1
    return nc


def _rne12(x):
    """Round fp32 mantissa to 11 explicit bits (RNE) — the float32r format."""
    v = np.ascontiguousarray(x, dtype=np.float32).view(np.uint32).astype(np.uint64)
    half = np.uint64(0x7FF) + ((v >> np.uint64(12)) & np.uint64(1))
    out = ((v + half) & np.uint64(0xFFFFF000)).astype(np.uint32)
    return out.view(np.float32)


def _build(reps=1, phases="A"):
    nc = bass.Bass()

    xt_in = nc.dram_tensor("xt", [FEAT, NTOT], BF16, kind="ExternalInput")
    xc_in = nc.dram_tensor("xc", [128, NTOT], F32R, kind="ExternalInput")
    wv_in = nc.dram_tensor("wv", [128, FEAT], BF16, kind="ExternalInput")
    w1t_in = nc.dram_tensor("w1t", [128, D], F32R, kind="ExternalInput")
    b1_in = nc.dram_tensor("b1", [D, 1], F32, kind="ExternalInput")
    w2dr_in = nc.dram_tensor("w2dr", [D, 2, N_SEQ], F8, kind="ExternalInput")
    wot_in = nc.dram_tensor("wot", [128, NC * FEAT], BF16, kind="ExternalInput")
    sel_in = nc.dram_tensor("sel", [2, 128], F32R, kind="ExternalInput")
    out_ext = nc.dram_tensor("out", [NBLK, FEAT], F32, kind="ExternalOutput")

    with tile.TileContext(nc) as tc, ExitStack() as ctx:
        wp = ctx.enter_context(tc.tile_pool(name="wp", bufs=1))
        dram = ctx.enter_context(tc.tile_pool(name="dram", bufs=1, space="DRAM"))

        # ---- resident weights/constants -------------------------------
        wv = wp.tile([128, FEAT], BF16)
        nc.sync.dma_start(wv[:], wv_in[:])
        w1t = wp.tile([128, D], F32R)           # W1.T stacked twice
        nc.sync.dma_start(w1t[:], w1t_in[:])
        b1t = wp.tile([D, 1], F32)
        nc.sync.dma_start(b1t[:], b1_in[:])
        w2dr = wp.tile([D, 2, N_SEQ], F8)       # 8*W2.T k-tiled; [0,1,:] = 8*b2
        nc.sync.dma_start(w2dr[:], w2dr_in[:])
        xc = wp.tile([128, NTOT], F32R)
        nc.sync.dma_start(xc[:], xc_in[:])
        sel = wp.tile([2, 128], F32R)
        nc.sync.dma_start(sel[:], sel_in[:])
        wot = wp.tile([128, NC * FEAT], BF16)
        nc.sync.dma_start(wot[:], wot_in[:])

        # double-buffered by rep parity: vh[p][b], hidT p -> hid_all[2p+h]
        vh = [
            [wp.tile([128, MC, 132], BF16, name=f"vh{p}{b}", tag=f"vh{p}{b}") for b in range(B)]
            for p in range(2)
        ]
        for p in range(2):
            for b in range(B):
                nc.vector.memset(vh[p][b][:, :, 64:65], 1.0)
                nc.vector.memset(vh[p][b][:, :, 130:131], 1.0)
        xfr = [
            [wp.tile([128, N_SEQ], BF16, name=f"xfr{g}_{f}", tag=f"xfr{g}_{f}") for f in range(8)]
            for g in range(2)
        ]
        for g in range(2):
            for f in range(8):
                nc.sync.dma_start(
                    xfr[g][f][:],
                    xt_in[f * 128 : (f + 1) * 128, g * N_SEQ : (g + 1) * N_SEQ],
                )
        hid_all = [wp.tile([D, 2, NTOT], F8, name=f"hidT{i}", tag=f"hidT{i}") for i in range(4)]
        for i in range(4):
            nc.vector.memset(hid_all[i][:, 1, :], 0.0)
            nc.vector.memset(hid_all[i][0:1, 1, :], 1.0)

        def emit_hid_block(pvp, par, nb):
            """hid MLP for both heads at block nb into parity `par`."""
            for h in range(2):
                ph = pvp.tile([128, NBLK], F32, tag=f"pv{nb % 2}", name="ph")
                nc.tensor.matmul(
                    ph[0:D, :],
                    w1t[h * D : (h + 1) * D, :],
                    xc[h * D : (h + 1) * D, nb * NBLK : (nb + 1) * NBLK],
                    start=True,
                    stop=True,
                    skip_group_check=True,
                )
                nc.scalar.activation(
                    hid_all[2 * par + h][:, 0, nb * NBLK : (nb + 1) * NBLK],
                    ph[0:D, :],
                    AF.Relu,
                    bias=b1t[:],
                )

        def emit_p1_chunks(pvp, par, j0, n):
            """value-projection m-chunks [j0, j0+n) into parity `par`.
            Global chunk j: batch j//16, local chunk j%16."""
            for j in range(j0, j0 + n):
                g, jl = j // MC, j % MC
                pv = pvp.tile([128, 128], F32, tag=f"pv{j % 2}", name="pv")
                for f in range(8):
                    nc.tensor.matmul(
                        pv[:],
                        xfr[g][f][:, jl * 128 : (jl + 1) * 128],
                        wv[:, f * 128 : (f + 1) * 128],
                        start=(f == 0),
                        stop=(f == 7),
                        skip_group_check=True,
                    )
                nc.vector.tensor_copy(vh[par][g][:, jl, 0:D], pv[:, 0:D])
                nc.vector.tensor_copy(vh[par][g][:, jl, 66 : 66 + D], pv[:, D:128])

        def emit_p4_norm(rp, awp, pvp, actw, a2a_recv, s):
            """P4 stage: normalize shard s of a prior rep's receives."""
            sums = rp.tile([2, NBLK], BF16, name="sums")
            nc.sync.dma_start(sums[0:1, :], a2a_recv[0][s, D : D + 1, :])
            nc.sync.dma_start(sums[1:2, :], a2a_recv[1][s, D : D + 1, :])
            raw = rp.tile([128, NBLK], BF16, name="raw")
            nc.sync.dma_start(raw[0:D, :], a2a_recv[0][s, 0:D, :])
            nc.sync.dma_start(raw[D:128, :], a2a_recv[1][s, 0:D, :])
            rcps_f = rp.tile([2, NBLK], F32, name="rcps_f")
            nc.vector.reciprocal(rcps_f[:], sums[:])
            rcps = rp.tile([2, NBLK], F32R, name="rcps")
            nc.vector.tensor_copy(rcps[:], rcps_f[:])
            pb = pvp.tile([128, NBLK], F32, tag=f"pv{s % 2}", name="pb")
            nc.tensor.matmul(
                pb[:], sel[:], rcps[:], start=True, stop=True,
                skip_group_check=True,
            )
            nc.vector.tensor_mul(actw[s][:], raw[:], pb[:])

        def emit_p4_proj(pvp, obp, actw, obs, th):
            """P4 stage: output-projection chain th (t = th//2, half = th%2)."""
            t, half = th // 2, th % 2
            pw = pvp.tile([128, 512], F32, tag=f"pv{th % 2}", name="pw")
            for s in range(NC):
                nc.tensor.matmul(
                    pw[:],
                    actw[s][:, t * 128 : (t + 1) * 128],
                    wot[:, s * FEAT + half * 512 : s * FEAT + (half + 1) * 512],
                    start=(s == 0),
                    stop=(s == NC - 1),
                    skip_group_check=True,
                )
            if half == 0:
                obs[t] = obp.tile([128, FEAT], F32, name="ob")
            nc.vector.tensor_copy(obs[t][:, half * 512 : (half + 1) * 512], pw[:])
            if half == 1:
                nc.sync.dma_start(out_ext[t * 128 : (t + 1) * 128, :], obs[t][:])

        def emit_p4(a2a_recv):
            """normalize + output projection for one rep's A2A receives."""
            with ExitStack() as c4:
                rp = c4.enter_context(tc.tile_pool(name="rp", bufs=6))
                psb = c4.enter_context(tc.tile_pool(name="psb", bufs=2, space="PSUM"))
                awp = c4.enter_context(tc.tile_pool(name="awp", bufs=1))
                psw = c4.enter_context(tc.tile_pool(name="psw", bufs=3, space="PSUM"))
                obp = c4.enter_context(tc.tile_pool(name="obp", bufs=3))

                actw = [awp.tile([128, NBLK], BF16, name=f"aw{s_}", tag=f"aw{s_}") for s_ in range(NC)]
                for s in range(NC):
                    sums = rp.tile([2, NBLK], BF16)
                    nc.sync.dma_start(sums[0:1, :], a2a_recv[0][s, D : D + 1, :])
                    nc.sync.dma_start(sums[1:2, :], a2a_recv[1][s, D : D + 1, :])
                    raw = rp.tile([128, NBLK], BF16)
                    nc.sync.dma_start(raw[0:D, :], a2a_recv[0][s, 0:D, :])
                    nc.sync.dma_start(raw[D:128, :], a2a_recv[1][s, 0:D, :])
                    rcps_f = rp.tile([2, NBLK], F32)
                    nc.vector.reciprocal(rcps_f[:], sums[:])
                    rcps = rp.tile([2, NBLK], F32R)
                    nc.vector.tensor_copy(rcps[:], rcps_f[:])
                    pb = psb.tile([128, NBLK], F32)
                    nc.tensor.matmul(
                        pb[:], sel[:], rcps[:], start=True, stop=True,
                        skip_group_check=True,
                    )
                    nc.vector.tensor_mul(actw[s][:], raw[:], pb[:])

                for t in range(NBLK // 128):
                    p0 = psw.tile([128, 512], F32, tag="pw0")
                    p1 = psw.tile([128, 512], F32, tag="pw1")
                    for s in range(NC):
                        nc.tensor.matmul(
                            p0[:],
                            actw[s][:, t * 128 : (t + 1) * 128],
                            wot[:, s * FEAT : s * FEAT + 512],
                            start=(s == 0),
                            stop=(s == NC - 1),
                            skip_group_check=True,
                        )
                        nc.tensor.matmul(
                            p1[:],
                            actw[s][:, t * 128 : (t + 1) * 128],
                            wot[:, s * FEAT + 512 : (s + 1) * FEAT],
                            start=(s == 0),
                            stop=(s == NC - 1),
                            skip_group_check=True,
                        )
                    ob = obp.tile([128, FEAT], F32)
                    nc.vector.tensor_copy(ob[:, 0:512], p0[:])
                    nc.vector.tensor_copy(ob[:, 512:1024], p1[:])
                    nc.sync.dma_start(out_ext[t * 128 : (t + 1) * 128, :], ob[:])

        # ---- prologue: hid + value projection for rep 0 (parity 0) ----
        with ExitStack() as c0:
            pvp0 = c0.enter_context(tc.tile_pool(name="pvp0", bufs=1, space="PSUM"))
            for nb in range(NB):
                emit_hid_block(pvp0, 0, nb)
            emit_p1_chunks(pvp0, 0, 0, 2 * MC)

        pending = []
        with ExitStack() as c2:
            # PSUM budget (8 banks):
            #   pvp: 2 tags x 2KB (hid ph + value accs + P4 pb/pw)  = 2
            #   psl: 2 bufs x 4KB (double-wide DR logits)           = 4
            #   pso: 2 bufs x 2KB (S@V accumulators)                = 2
            pvp = c2.enter_context(tc.tile_pool(name="pvp", bufs=1, space="PSUM"))
            psl = c2.enter_context(tc.tile_pool(name="psl", bufs=2, space="PSUM"))
            pso = c2.enter_context(tc.tile_pool(name="pso", bufs=2, space="PSUM"))
            ep = c2.enter_context(tc.tile_pool(name="ep", bufs=8))
            op = c2.enter_context(tc.tile_pool(name="op", bufs=4))
            rp = c2.enter_context(tc.tile_pool(name="rp", bufs=1))
            awp = c2.enter_context(tc.tile_pool(name="awp", bufs=1))
            obp = c2.enter_context(tc.tile_pool(name="obp", bufs=1))
            for _rep in range(reps):
                par = _rep % 2
                a2a_send = [dram.tile([NC, 65, NBLK], BF16, name=f"snd{h}_{_rep}") for h in range(2)]
                a2a_recv = [dram.tile([NC, 65, NBLK], BF16, name=f"rcv{h}_{_rep}") for h in range(2)]

                fill = _rep + 1 < reps  # emit next rep's hid/P1 interleaved
                nxt = 1 - par
                prev = pending.pop() if (pending and phases not in ("1", "2", "3")) else None
                actw = (
                    [awp.tile([128, NBLK], BF16, name=f"aw{s_}", tag=f"aw{s_}") for s_ in range(NC)]
                    if prev is not None
                    else None
                )
                obs = [None] * (NBLK // 128)
                blocks = [(h, nb) for h in range(2) for nb in range(NB)]

                def emit_logits_exp(h, nb):
                    hidT = hid_all[2 * par + h]
                    # quarter-size exp tiles (bufs=8, two blocks in flight):
                    # logits+exp for block k+1 are emitted before S@V of
                    # block k so the Activation engine never starves
                    eqs = []
                    for qt in range(4):
                        eq = ep.tile([128, 4 * NBLK], BF16, name="expTq", tag="expTq")
                        eqs.append(eq)
                        if phases == "E":
                            nc.vector.memset(eq[:, :], 1.0)
                        for jj in range(0, 4, 2):
                            pl = psl.tile([128, 2 * NBLK], F32)
                            for q in range(2):
                                j = qt * 4 + jj + q
                                nc.tensor.matmul(
                                    pl[:, q * NBLK : (q + 1) * NBLK],
                                    w2dr[:, :, j * 128 : (j + 1) * 128],
                                    hidT[:, :, nb * NBLK : (nb + 1) * NBLK],
                                    start=True,
                                    stop=True,
                                    perf_mode=mybir.MatmulPerfMode.DoubleRow,
                                    skip_group_check=True,
                                )
                            if phases != "E":
                                nc.scalar.activation(
                                    eq[:, jj * NBLK : (jj + 2) * NBLK],
                                    pl[:],
                                    AF.Exp,
                                    scale=1.0 / W2SCALE,
                                )
                    return eqs

                eqs_next = emit_logits_exp(*blocks[0])
                for k, (h, nb) in enumerate(blocks):
                    b = nb // (NB // B)
                    eqs = eqs_next
                    if k + 1 < len(blocks):
                        eqs_next = emit_logits_exp(*blocks[k + 1])
                    po = pso.tile([65, NBLK], F32)
                    if phases == "X":
                        nc.vector.memset(po[:], 1.0)
                    else:
                        for j in range(MC):
                            nc.tensor.matmul(
                                po[:],
                                vh[par][b][:, j, h * 66 : h * 66 + 65],
                                eqs[j // 4][:, (j % 4) * NBLK : (j % 4 + 1) * NBLK],
                                start=(j == 0),
                                stop=(j == MC - 1),
                                skip_group_check=True,
                            )
                    ot = op.tile([65, NBLK], BF16)
                    nc.vector.tensor_copy(ot[:], po[:])
                    nc.sync.dma_start(a2a_send[h][nb], ot[:])

                    # ---- pipelined fill for the next rep --------------
                    if fill:
                        if h == 0:
                            emit_p1_chunks(pvp, nxt, 4 * nb, 4)
                        else:
                            emit_hid_block(pvp, nxt, nb)
                    # ---- P4 of the previous rep, woven into h=1 so the
                    # A2A gets a half-rep to land -----------------------
                    if prev is not None and h == 1:
                        if nb < 4:
                            emit_p4_norm(rp, awp, pvp, actw, prev, 2 * nb)
                            emit_p4_norm(rp, awp, pvp, actw, prev, 2 * nb + 1)
                        else:
                            emit_p4_proj(pvp, obp, actw, obs, 2 * (nb - 4))
                            emit_p4_proj(pvp, obp, actw, obs, 2 * (nb - 4) + 1)

                    # fire this head's exchange as soon as its blocks are out
                    if nb == NB - 1 and phases not in ("1", "2"):
                        nc.gpsimd.collective_compute(
                            "AllToAll",
                            mybir.AluOpType.bypass,
                            ins=[a2a_send[h][:].opt()],
                            outs=[a2a_recv[h][:].opt()],
                            replica_groups=[list(range(NC))],
                        )

                if phases in ("1", "2", "3"):
                    continue

                # ---- P4 runs one rep behind, woven into the next rep's
                # blocks; the final rep's P4 drains after this scope ----
                pending.append(a2a_recv)

        for recv in pending:
            if phases not in ("1", "2", "3"):
                emit_p4(recv)

    _split_sem_waits(nc)
    return nc


_CACHE = {}


def _get_program(reps=1, phases="A"):
    key = ("nc", reps, phases)
    if key not in _CACHE:
        _CACHE[key] = _build(reps, phases)
    return _CACHE[key]


def kernel(x, W1, b1, W2, b2, Wv, Wo, _run_kwargs=None):
    x = np.asarray(x, dtype=np.float32)
    W1 = np.asarray(W1, dtype=np.float32)
    b1 = np.asarray(b1, dtype=np.float32)
    W2 = np.asarray(W2, dtype=np.float32)
    b2 = np.asarray(b2, dtype=np.float32)
    Wv = np.asarray(Wv, dtype=np.float32)
    Wo = np.asarray(Wo, dtype=np.float32)

    xr = x.reshape(NTOT, FEAT)
    xt_f = np.ascontiguousarray(xr.T)                          # [1024, 4096] f32
    xt_bf = xt_f.astype(ml_dtypes.bfloat16)
    xt_r = _rne12(xt_f)                                        # f32r for xc slices
    w1t = _rne12(np.concatenate([W1.T, W1.T], axis=0))         # [128, 64]
    # DoubleRow W2: k-tile 0 = 8*W2.T [64, 2048]; k-tile 1 row 0 = 8*b2
    w2dr = np.zeros((D, 2, N_SEQ), dtype=np.float32)
    w2dr[:, 0, :] = W2SCALE * W2.T
    w2dr[0, 1, :] = W2SCALE * b2
    w2dr = w2dr.astype(ml_dtypes.float8_e4m3)
    wot = (
        Wo.T.reshape(NC, 128, FEAT).transpose(1, 0, 2).reshape(128, NC * FEAT)
    ).astype(ml_dtypes.bfloat16)
    b1c = np.ascontiguousarray(b1.reshape(D, 1))
    sel_h = np.zeros((2, 128), dtype=np.float32)
    sel_h[0, :D] = 1.0
    sel_h[1, D:] = 1.0

    in_maps = []
    for c in range(NC):
        wv_c_blocks = Wv[c * 128 : (c + 1) * 128, :]           # [128 d, 1024 f]
        wv_c = np.concatenate(
            [wv_c_blocks[:, f * 128 : (f + 1) * 128].T for f in range(8)], axis=1
        ).astype(ml_dtypes.bfloat16)                           # [128 f, 1024] col-block f
        in_maps.append(
            {
                "xt": xt_bf,
                "xc": np.ascontiguousarray(xt_r[c * 128 : (c + 1) * 128, :]),
                "wv": wv_c,
                "w1t": w1t,
                "b1": b1c,
                "w2dr": w2dr,
                "wot": wot,
                "sel": sel_h,
            }
        )

    import os
    nc = _get_program(
        int(os.environ.get("KERNEL_REPS", "1")), os.environ.get("KERNEL_PHASES", "A")
    )
    res = run_bass_kernel_spmd(
        nc, in_maps, list(range(NC)), **(_run_kwargs or {})
    )
    out = np.concatenate([res.results[c]["out"] for c in range(NC)], axis=0)
    if _run_kwargs:
        kernel.last_results = res
    return out.reshape(B, N_SEQ, FEAT)


# revision 18
# speedup vs baseline: 44.7557x; 44.7557x over previous
"""MultiHeadDenseAttention on 8 Trainium2 NeuronCores.

Head-sharded tensor parallelism: each core computes 2 of 16 heads
(value projection slice, per-head MLP attention logits, softmax, S@V),
then an AllToAll exchanges head-blocks for row-blocks so each core
computes the output projection for its 512 rows with the full Wo.

Schedule (v5): the Activation engine (exp of 16.8M logits/core) is the
hard floor, so everything else is software-pipelined around it:
  - logits matmul in fp8e4m3 DoubleRow mode (2 k-tiles of 64 contracted
    per pass, 0.5 cycles/row): hid stored fp8 [64, 2, 4096], k-tile 1 =
    [ones (fused b2); zeros]; W2 host-scaled by 8, un-scaled in exp.
  - value projection directly in [m, d] layout (lhsT = x chunk, rhs =
    Wv.T block, PSUM accumulation over the 8 feature chunks) in bf16.
  - value/hid for rep r+1 are emitted interleaved between rep r's P2
    blocks (double-buffered vh/hidT by rep parity), so the exp stream
    never waits on them.
  - A2A payload in bf16; P4 (normalize + output projection) of rep r-1
    is emitted between rep r and r+1, hiding each collective behind a
    full rep of compute.

Layouts (per core c, heads 2c / 2c+1):
  xt    [1024, 4096] bf16  x.reshape(4096,1024).T  (feat on partitions)
  xc    [128, 4096] f32r   xt rows [128c, 128c+128) (this core's head cols)
  wv    [128, 1024] bf16   col block f = Wv.T[128f:+128, 128c:+128]
  vh[p][b] [128, 16, 132] bf16  per m-chunk: [64 h0 | one | pad | 64 h1 | one | pad]
  hidT  [64, 2, 4096] fp8  k-tile 0 = relu(W1@x+b1), k-tile 1 = ones/zeros
  w2dr  [64, 2, 2048] fp8  k-tile 0 = 8*W2.T, k-tile 1 row 0 = 8*b2
  S@V: po[65, 512] = vh_aug.T @ expT  (row 64 = sum of exp)
  A2A [8, 65, 512] bf16 x2 heads, normalize after exchange, out = act @ Wo.T.
"""

import sys

if "/opt/trn_rl_repo" not in sys.path:
    sys.path.insert(0, "/opt/trn_rl_repo")

from contextlib import ExitStack

import numpy as np
import ml_dtypes

import bass_rust
import concourse.bass as bass
import concourse.tile as tile
from concourse import mybir
from concourse.bass_utils import run_bass_kernel_spmd

F32 = mybir.dt.float32
F32R = mybir.dt.float32r
BF16 = mybir.dt.bfloat16
F8 = mybir.dt.float8e4
AF = mybir.ActivationFunctionType

NC = 8            # cores
B = 2             # batch
N_SEQ = 2048      # seq len == max_seq_len (m)
FEAT = 1024
H = 16            # heads
D = 64            # head dim
NTOT = B * N_SEQ  # 4096 flattened rows
NBLK = 512        # n-block size
NB = NTOT // NBLK # 8 n-blocks (== A2A shards == cores)
MC = N_SEQ // 128 # 16 m-chunks per batch
W2SCALE = 8.0     # host-side W2 scaling into fp8 range; undone in exp


def _split_sem_waits(nc, limit=1):
    """Walrus rejects instructions with more than ~1 sync wait; move the
    excess onto NOPs on the same engine inserted immediately before."""
    blocks = {}
    for f in nc.m.functions:
        for bb in f.blocks:
            blocks[bb.name] = bb
    for bb in blocks.values():
        i = 0
        while i < len(bb.instructions):
            inst = bb.instructions[i]
            si = inst.sync_info
            if si is not None and si.on_wait and len(si.on_wait) > limit:
                waits = list(si.on_wait)
                chunks = [waits[j : j + limit] for j in range(0, len(waits), limit)]
                si.on_wait = chunks[-1]
                engine = nc.engines[inst.engine]
                for chunk in chunks[:-1]:
                    d = engine.nop(nofuse=True, hint="wait_split")
                    dinst = d.ins if hasattr(d, "ins") else d
                    for ob in blocks.values():
                        if ob.instructions and ob.instructions[-1] is dinst:
                            ob.instructions.pop()
                            break
                    dinst.sync_info = bass_rust.SyncInfo(on_wait=chunk, on_update=[])
                    bb.instructions.insert(i, dinst)
                    i += 1
            i += 1
    return nc


def _rne12(x):
    """Round fp32 mantissa to 11 explicit bits (RNE) — the float32r format."""
    v = np.ascontiguousarray(x, dtype=np.float32).view(np.uint32).astype(np.uint64)
    half = np.uint64(0x7FF) + ((v >> np.uint64(12)) & np.uint64(1))
    out = ((v + half) & np.uint64(0xFFFFF000)).astype(np.uint32)
    return out.view(np.float32)


def _build(reps=1, phases="A"):
    nc = bass.Bass()

    xt_in = nc.dram_tensor("xt", [FEAT, NTOT], BF16, kind="ExternalInput")
    xc_in = nc.dram_tensor("xc", [128, NTOT], F32R, kind="ExternalInput")
    wv_in = nc.dram_tensor("wv", [128, FEAT], BF16, kind="ExternalInput")
    w1t_in = nc.dram_tensor("w1t", [128, D], F32R, kind="ExternalInput")
    b1_in = nc.dram_tensor("b1", [D, 1], F32, kind="ExternalInput")
    w2dr_in = nc.dram_tensor("w2dr", [D, 2, N_SEQ], F8, kind="ExternalInput")
    wot_in = nc.dram_tensor("wot", [128, NC * FEAT], BF16, kind="ExternalInput")
    sel_in = nc.dram_tensor("sel", [2, 128], F32R, kind="ExternalInput")
    out_ext = nc.dram_tensor("out", [NBLK, FEAT], F32, kind="ExternalOutput")

    with tile.TileContext(nc) as tc, ExitStack() as ctx:
        wp = ctx.enter_context(tc.tile_pool(name="wp", bufs=1))
        dram = ctx.enter_context(tc.tile_pool(name="dram", bufs=1, space="DRAM"))

        # ---- resident weights/constants -------------------------------
        wv = wp.tile([128, FEAT], BF16)
        nc.sync.dma_start(wv[:], wv_in[:])
        w1t = wp.tile([128, D], F32R)           # W1.T stacked twice
        nc.sync.dma_start(w1t[:], w1t_in[:])
        b1t = wp.tile([D, 1], F32)
        nc.sync.dma_start(b1t[:], b1_in[:])
        w2dr = wp.tile([D, 2, N_SEQ], F8)       # 8*W2.T k-tiled; [0,1,:] = 8*b2
        nc.sync.dma_start(w2dr[:], w2dr_in[:])
        xc = wp.tile([128, NTOT], F32R)
        nc.sync.dma_start(xc[:], xc_in[:])
        sel = wp.tile([2, 128], F32R)
        nc.sync.dma_start(sel[:], sel_in[:])
        wot = wp.tile([128, NC * FEAT], BF16)
        nc.sync.dma_start(wot[:], wot_in[:])

        # double-buffered by rep parity: vh[p][b], hidT p -> hid_all[2p+h]
        vh = [
            [wp.tile([128, MC, 132], BF16, name=f"vh{p}{b}", tag=f"vh{p}{b}") for b in range(B)]
            for p in range(2)
        ]
        for p in range(2):
            for b in range(B):
                nc.vector.memset(vh[p][b][:, :, 64:65], 1.0)
                nc.vector.memset(vh[p][b][:, :, 130:131], 1.0)
        xfr = [
            [wp.tile([128, N_SEQ], BF16, name=f"xfr{g}_{f}", tag=f"xfr{g}_{f}") for f in range(8)]
            for g in range(2)
        ]
        for g in range(2):
            for f in range(8):
                nc.sync.dma_start(
                    xfr[g][f][:],
                    xt_in[f * 128 : (f + 1) * 128, g * N_SEQ : (g + 1) * N_SEQ],
                )
        hid_all = [wp.tile([D, 2, NTOT], F8, name=f"hidT{i}", tag=f"hidT{i}") for i in range(4)]
        for i in range(4):
            nc.vector.memset(hid_all[i][:, 1, :], 0.0)
            nc.vector.memset(hid_all[i][0:1, 1, :], 1.0)

        def emit_hid_block(pvp, par, nb):
            """hid MLP for both heads at block nb into parity `par`."""
            for h in range(2):
                ph = pvp.tile([128, NBLK], F32, tag=f"pv{nb % 2}", name="ph")
                nc.tensor.matmul(
                    ph[0:D, :],
                    w1t[h * D : (h + 1) * D, :],
                    xc[h * D : (h + 1) * D, nb * NBLK : (nb + 1) * NBLK],
                    start=True,
                    stop=True,
                    skip_group_check=True,
                )
                # relu on DVE (dual-op tensor_scalar) to keep the ACT
                # engine free for the exp stream
                nc.vector.tensor_scalar(
                    hid_all[2 * par + h][:, 0, nb * NBLK : (nb + 1) * NBLK],
                    ph[0:D, :],
                    b1t[:],
                    0.0,
                    mybir.AluOpType.add,
                    mybir.AluOpType.max,
                )

        def emit_p1_chunks(pvp, par, j0, n):
            """value-projection m-chunks [j0, j0+n) into parity `par`.
            Global chunk j: batch j//16, local chunk j%16."""
            for j in range(j0, j0 + n):
                g, jl = j // MC, j % MC
                pv = pvp.tile([128, 128], F32, tag=f"pv{j % 2}", name="pv")
                for f in range(8):
                    nc.tensor.matmul(
                        pv[:],
                        xfr[g][f][:, jl * 128 : (jl + 1) * 128],
                        wv[:, f * 128 : (f + 1) * 128],
                        start=(f == 0),
                        stop=(f == 7),
                        skip_group_check=True,
                    )
                nc.vector.tensor_copy(vh[par][g][:, jl, 0:D], pv[:, 0:D])
                nc.vector.tensor_copy(vh[par][g][:, jl, 66 : 66 + D], pv[:, D:128])

        def emit_p4_norm(rp, awp, pvp, actw, a2a_recv, s):
            """P4 stage: normalize shard s of a prior rep's receives."""
            sums = rp.tile([2, NBLK], BF16, name="sums")
            nc.sync.dma_start(sums[0:1, :], a2a_recv[0][s, D : D + 1, :])
            nc.sync.dma_start(sums[1:2, :], a2a_recv[1][s, D : D + 1, :])
            raw = rp.tile([128, NBLK], BF16, name="raw")
            nc.sync.dma_start(raw[0:D, :], a2a_recv[0][s, 0:D, :])
            nc.sync.dma_start(raw[D:128, :], a2a_recv[1][s, 0:D, :])
            rcps_f = rp.tile([2, NBLK], F32, name="rcps_f")
            nc.vector.reciprocal(rcps_f[:], sums[:])
            rcps = rp.tile([2, NBLK], F32R, name="rcps")
            nc.vector.tensor_copy(rcps[:], rcps_f[:])
            pb = pvp.tile([128, NBLK], F32, tag=f"pv{s % 2}", name="pb")
            nc.tensor.matmul(
                pb[:], sel[:], rcps[:], start=True, stop=True,
                skip_group_check=True,
            )
            nc.vector.tensor_mul(actw[s][:], raw[:], pb[:])

        def emit_p4_proj(pvp, obp, actw, obs, th):
            """P4 stage: output-projection chain th (t = th//2, half = th%2)."""
            t, half = th // 2, th % 2
            pw = pvp.tile([128, 512], F32, tag=f"pv{th % 2}", name="pw")
            for s in range(NC):
                nc.tensor.matmul(
                    pw[:],
                    actw[s][:, t * 128 : (t + 1) * 128],
                    wot[:, s * FEAT + half * 512 : s * FEAT + (half + 1) * 512],
                    start=(s == 0),
                    stop=(s == NC - 1),
                    skip_group_check=True,
                )
            if half == 0:
                obs[t] = obp.tile([128, FEAT], F32, name="ob")
            nc.vector.tensor_copy(obs[t][:, half * 512 : (half + 1) * 512], pw[:])
            if half == 1:
                nc.sync.dma_start(out_ext[t * 128 : (t + 1) * 128, :], obs[t][:])

        def emit_p4(a2a_recv):
            """normalize + output projection for one rep's A2A receives."""
            with ExitStack() as c4:
                rp = c4.enter_context(tc.tile_pool(name="rp", bufs=6))
                psb = c4.enter_context(tc.tile_pool(name="psb", bufs=2, space="PSUM"))
                awp = c4.enter_context(tc.tile_pool(name="awp", bufs=1))
                psw = c4.enter_context(tc.tile_pool(name="psw", bufs=3, space="PSUM"))
                obp = c4.enter_context(tc.tile_pool(name="obp", bufs=3))

                actw = [awp.tile([128, NBLK], BF16, name=f"aw{s_}", tag=f"aw{s_}") for s_ in range(NC)]
                for s in range(NC):
                    sums = rp.tile([2, NBLK], BF16)
                    nc.sync.dma_start(sums[0:1, :], a2a_recv[0][s, D : D + 1, :])
                    nc.sync.dma_start(sums[1:2, :], a2a_recv[1][s, D : D + 1, :])
                    raw = rp.tile([128, NBLK], BF16)
                    nc.sync.dma_start(raw[0:D, :], a2a_recv[0][s, 0:D, :])
                    nc.sync.dma_start(raw[D:128, :], a2a_recv[1][s, 0:D, :])
                    rcps_f = rp.tile([2, NBLK], F32)
                    nc.vector.reciprocal(rcps_f[:], sums[:])
                    rcps = rp.tile([2, NBLK], F32R)
                    nc.vector.tensor_copy(rcps[:], rcps_f[:])
                    pb = psb.tile([128, NBLK], F32)
                    nc.tensor.matmul(
                        pb[:], sel[:], rcps[:], start=True, stop=True,
                        skip_group_check=True,
                    )
                    nc.vector.tensor_mul(actw[s][:], raw[:], pb[:])

                for t in range(NBLK // 128):
                    p0 = psw.tile([128, 512], F32, tag="pw0")
                    p1 = psw.tile([128, 512], F32, tag="pw1")
                    for s in range(NC):
                        nc.tensor.matmul(
                            p0[:],
                            actw[s][:, t * 128 : (t + 1) * 128],
                            wot[:, s * FEAT : s * FEAT + 512],
                            start=(s == 0),
                            stop=(s == NC - 1),
                            skip_group_check=True,
                        )
                        nc.tensor.matmul(
                            p1[:],
                            actw[s][:, t * 128 : (t + 1) * 128],
                            wot[:, s * FEAT + 512 : (s + 1) * FEAT],
                            start=(s == 0),
                            stop=(s == NC - 1),
                            skip_group_check=True,
                        )
                    ob = obp.tile([128, FEAT], F32)
                    nc.vector.tensor_copy(ob[:, 0:512], p0[:])
                    nc.vector.tensor_copy(ob[:, 512:1024], p1[:])
                    nc.sync.dma_start(out_ext[t * 128 : (t + 1) * 128, :], ob[:])

        # ---- prologue: hid + value projection for rep 0 (parity 0) ----
        with ExitStack() as c0:
            pvp0 = c0.enter_context(tc.tile_pool(name="pvp0", bufs=1, space="PSUM"))
            for nb in range(NB):
                emit_hid_block(pvp0, 0, nb)
            emit_p1_chunks(pvp0, 0, 0, 2 * MC)

        pending = []
        eqs_carry = None
        with ExitStack() as c2:
            # PSUM budget (8 banks):
            #   pvp: 2 tags x 2KB (hid ph + value accs + P4 pb/pw)  = 2
            #   psl: 2 bufs x 4KB (double-wide DR logits)           = 4
            #   pso: 2 bufs x 2KB (S@V accumulators)                = 2
            pvp = c2.enter_context(tc.tile_pool(name="pvp", bufs=1, space="PSUM"))
            psl = c2.enter_context(tc.tile_pool(name="psl", bufs=2, space="PSUM"))
            pso = c2.enter_context(tc.tile_pool(name="pso", bufs=2, space="PSUM"))
            ep = c2.enter_context(tc.tile_pool(name="ep", bufs=8))
            op = c2.enter_context(tc.tile_pool(name="op", bufs=4))
            rp = c2.enter_context(tc.tile_pool(name="rp", bufs=1))
            awp = c2.enter_context(tc.tile_pool(name="awp", bufs=1))
            obp = c2.enter_context(tc.tile_pool(name="obp", bufs=1))
            for _rep in range(reps):
                par = _rep % 2
                a2a_send = [dram.tile([NC, 65, NBLK], BF16, name=f"snd{h}_{_rep}") for h in range(2)]
                a2a_recv = [dram.tile([NC, 65, NBLK], BF16, name=f"rcv{h}_{_rep}") for h in range(2)]

                fill = _rep + 1 < reps  # emit next rep's hid/P1 interleaved
                nxt = 1 - par
                prev = pending.pop() if (pending and phases not in ("1", "2", "3")) else None
                actw = (
                    [awp.tile([128, NBLK], BF16, name=f"aw{s_}", tag=f"aw{s_}") for s_ in range(NC)]
                    if prev is not None
                    else None
                )
                obs = [None] * (NBLK // 128)
                blocks = [(h, nb) for h in range(2) for nb in range(NB)]

                def emit_logits_exp(h, nb, p=None):
                    hidT = hid_all[2 * (par if p is None else p) + h]
                    # quarter-size exp tiles (bufs=8, two blocks in flight):
                    # logits+exp for block k+1 are emitted before S@V of
                    # block k so the Activation engine never starves
                    eqs = []
                    for qt in range(4):
                        eq = ep.tile([128, 4 * NBLK], BF16, name="expTq", tag="expTq")
                        eqs.append(eq)
                        if phases == "E":
                            nc.vector.memset(eq[:, :], 1.0)
                        for jj in range(0, 4, 2):
                            pl = psl.tile([128, 2 * NBLK], F32)
                            for q in range(2):
                                j = qt * 4 + jj + q
                                nc.tensor.matmul(
                                    pl[:, q * NBLK : (q + 1) * NBLK],
                                    w2dr[:, :, j * 128 : (j + 1) * 128],
                                    hidT[:, :, nb * NBLK : (nb + 1) * NBLK],
                                    start=True,
                                    stop=True,
                                    perf_mode=mybir.MatmulPerfMode.DoubleRow,
                                    skip_group_check=True,
                                )
                            if phases != "E":
                                nc.scalar.activation(
                                    eq[:, jj * NBLK : (jj + 2) * NBLK],
                                    pl[:],
                                    AF.Exp,
                                    scale=1.0 / W2SCALE,
                                )
                    return eqs

                if eqs_carry is None:
                    eqs_carry = emit_logits_exp(*blocks[0])
                for k, (h, nb) in enumerate(blocks):
                    b = nb // (NB // B)
                    eqs = eqs_carry
                    if k + 1 < len(blocks):
                        eqs_carry = emit_logits_exp(*blocks[k + 1])
                    elif fill:
                        # cross-rep lookahead: next rep's first logits/exp
                        # (its hidT parity was filled during this rep)
                        eqs_carry = emit_logits_exp(0, 0, nxt)
                    else:
                        eqs_carry = None
                    po = pso.tile([65, NBLK], F32)
                    if phases == "X":
                        nc.vector.memset(po[:], 1.0)
                    else:
                        for j in range(MC):
                            nc.tensor.matmul(
                                po[:],
                                vh[par][b][:, j, h * 66 : h * 66 + 65],
                                eqs[j // 4][:, (j % 4) * NBLK : (j % 4 + 1) * NBLK],
                                start=(j == 0),
                                stop=(j == MC - 1),
                                skip_group_check=True,
                            )
                    ot = op.tile([65, NBLK], BF16)
                    nc.vector.tensor_copy(ot[:], po[:])
                    nc.sync.dma_start(a2a_send[h][nb], ot[:])

                    # ---- pipelined fill for the next rep --------------
                    if fill:
                        if h == 0:
                            emit_p1_chunks(pvp, nxt, 4 * nb, 4)
                        else:
                            emit_hid_block(pvp, nxt, nb)
                    # ---- P4 of the previous rep, woven into h=1 so the
                    # A2A gets a half-rep to land -----------------------
                    if prev is not None and h == 1:
                        if nb < 4:
                            emit_p4_norm(rp, awp, pvp, actw, prev, 2 * nb)
                            emit_p4_norm(rp, awp, pvp, actw, prev, 2 * nb + 1)
                        else:
                            emit_p4_proj(pvp, obp, actw, obs, 2 * (nb - 4))
                            emit_p4_proj(pvp, obp, actw, obs, 2 * (nb - 4) + 1)

                    # fire this head's exchange as soon as its blocks are out
                    if nb == NB - 1 and phases not in ("1", "2"):
                        nc.gpsimd.collective_compute(
                            "AllToAll",
                            mybir.AluOpType.bypass,
                            ins=[a2a_send[h][:].opt()],
                            outs=[a2a_recv[h][:].opt()],
                            replica_groups=[list(range(NC))],
                        )

                if phases in ("1", "2", "3"):
                    continue

                # ---- P4 runs one rep behind, woven into the next rep's
                # blocks; the final rep's P4 drains after this scope ----
                pending.append(a2a_recv)

        for recv in pending:
            if phases not in ("1", "2", "3"):
                emit_p4(recv)

    _split_sem_waits(nc)
    return nc


_CACHE = {}


def _get_program(reps=1, phases="A"):
    key = ("nc", reps, phases)
    if key not in _CACHE:
        _CACHE[key] = _build(reps, phases)
    return _CACHE[key]


def kernel(x, W1, b1, W2, b2, Wv, Wo, _run_kwargs=None):
    x = np.asarray(x, dtype=np.float32)
    W1 = np.asarray(W1, dtype=np.float32)
    b1 = np.asarray(b1, dtype=np.float32)
    W2 = np.asarray(W2, dtype=np.float32)
    b2 = np.asarray(b2, dtype=np.float32)
    Wv = np.asarray(Wv, dtype=np.float32)
    Wo = np.asarray(Wo, dtype=np.float32)

    xr = x.reshape(NTOT, FEAT)
    xt_f = np.ascontiguousarray(xr.T)                          # [1024, 4096] f32
    xt_bf = xt_f.astype(ml_dtypes.bfloat16)
    xt_r = _rne12(xt_f)                                        # f32r for xc slices
    w1t = _rne12(np.concatenate([W1.T, W1.T], axis=0))         # [128, 64]
    # DoubleRow W2: k-tile 0 = 8*W2.T [64, 2048]; k-tile 1 row 0 = 8*b2
    w2dr = np.zeros((D, 2, N_SEQ), dtype=np.float32)
    w2dr[:, 0, :] = W2SCALE * W2.T
    w2dr[0, 1, :] = W2SCALE * b2
    w2dr = w2dr.astype(ml_dtypes.float8_e4m3)
    wot = (
        Wo.T.reshape(NC, 128, FEAT).transpose(1, 0, 2).reshape(128, NC * FEAT)
    ).astype(ml_dtypes.bfloat16)
    b1c = np.ascontiguousarray(b1.reshape(D, 1))
    sel_h = np.zeros((2, 128), dtype=np.float32)
    sel_h[0, :D] = 1.0
    sel_h[1, D:] = 1.0

    in_maps = []
    for c in range(NC):
        wv_c_blocks = Wv[c * 128 : (c + 1) * 128, :]           # [128 d, 1024 f]
        wv_c = np.concatenate(
            [wv_c_blocks[:, f * 128 : (f + 1) * 128].T for f in range(8)], axis=1
        ).astype(ml_dtypes.bfloat16)                           # [128 f, 1024] col-block f
        in_maps.append(
            {
                "xt": xt_bf,
                "xc": np.ascontiguousarray(xt_r[c * 128 : (c + 1) * 128, :]),
                "wv": wv_c,
                "w1t": w1t,
                "b1": b1c,
                "w2dr": w2dr,
                "wot": wot,
                "sel": sel_h,
            }
        )

    import os
    nc = _get_program(
        int(os.environ.get("KERNEL_REPS", "1")), os.environ.get("KERNEL_PHASES", "A")
    )
    res = run_bass_kernel_spmd(
        nc, in_maps, list(range(NC)), **(_run_kwargs or {})
    )
    out = np.concatenate([res.results[c]["out"] for c in range(NC)], axis=0)
    if _run_kwargs:
        kernel.last_results = res
    return out.reshape(B, N_SEQ, FEAT)
